# revision 3
# baseline (speedup 1.0000x reference)
"""Kernel builder for causal self-attention (RoPE + parameter-free RMSNorm on Q/K).

Sharding: 8 cores = 4 batch x 2 head-groups (8 heads each). Each core computes
its batch element's attention for its 8 heads plus the partial output
projection; host sums the two head-group partials per batch element.

Per-core device layout (D=64, 8 heads):
  Q^T / K^T stored as [128, 4, T]: col j = 128*cc + p,
     cc = 2*(h//4) + half, p = 32*(h%4) + r,  (d = 32*half + r)
  V stored with a ones column per head: [128, T//128, 8*65]; the ones column
  makes the PV matmul also accumulate the softmax denominator (row 64).
  Scores computed transposed: S^T[tk, tq] per head via K=32 row-tiled matmuls;
  softmax runs without max-subtraction (RMS-normed q,k bound |s| <= 8);
  the denominator division folds in before the output projection.
"""

import sys

import numpy as np

for _p in ("/opt/trn_rl_repo",):
    if _p not in sys.path:
        sys.path.insert(0, _p)

import concourse.bass as bass
import concourse.mybir as mybir
import concourse.tile as tile
from concourse import bacc

F32 = mybir.dt.float32
F32R = mybir.dt.float32r
AX = mybir.AluOpType
ACTF = mybir.ActivationFunctionType

D = 64
NH = 8          # heads per core
CH = NH * D     # 512 head channels per core
EPS = float(np.finfo(np.float32).eps)


def qk_col_perm():
    """perm[j] = plain column (64*h + d) stored at device column j."""
    perm = np.zeros(CH, dtype=np.int64)
    for h in range(NH):
        for half in range(2):
            for r in range(32):
                j = 128 * (2 * (h // 4) + half) + 32 * (h % 4) + r
                perm[j] = 64 * h + 32 * half + r
    return perm


def make_consts(T):
    """Host-side constant tensors fed as kernel inputs."""
    cs_d = D // 2
    inv_freq = 1.0 / (10000.0 ** (np.arange(cs_d, dtype=np.float64) / cs_d))
    freqs = np.outer(np.arange(T, dtype=np.float64), inv_freq)  # [T, 32]
    cosT = np.cos(freqs).astype(np.float32).T  # [32, T]
    sinT = np.sin(freqs).astype(np.float32).T
    COS = np.tile(cosT, (4, 1))  # [128, T]
    SIN = np.tile(sinT, (4, 1))
    # Boundary mask strip [128, 128]: MASK[p, j] = (p <= j)
    p = np.arange(128)[:, None]
    j = np.arange(128)[None, :]
    MASK = (p <= j).astype(np.float32)
    # SEL for ssq reduction: SEL_g[p, m] = 1 if m == 4*g + p//32  ([128, 8])
    SELA = np.zeros((128, 8), dtype=np.float32)
    SELB = np.zeros((128, 8), dtype=np.float32)
    for pp in range(128):
        SELA[pp, pp // 32] = 1.0
        SELB[pp, 4 + pp // 32] = 1.0
    SELTA = SELA.T.copy()
    SELTB = SELB.T.copy()
    ONESF = np.ones((128, 128), dtype=np.float32)
    return dict(COS=COS, SIN=SIN, MASK=MASK, SELA=SELA, SELB=SELB,
                SELTA=SELTA, SELTB=SELTB, ONESF=ONESF)


def make_core_inputs(x_b, Wq_s, Wk_s, Wv_s, Wo_s, consts):
    """x_b [T, CIN]; W*_s are this core's shards: Wq/Wk/Wv [CIN, 512] (plain
    column order 64h+d), Wo_s [512, COUT]. Returns the kernel input map."""
    perm = qk_col_perm()
    return dict(
        xT=np.ascontiguousarray(x_b.T),
        Wq=np.ascontiguousarray(Wq_s[:, perm]),
        Wk=np.ascontiguousarray(Wk_s[:, perm]),
        Wv=np.ascontiguousarray(Wv_s),
        Wo=np.ascontiguousarray(Wo_s),
        **{k: np.ascontiguousarray(v) for k, v in consts.items()},
    )


def build_nc(T, CIN, COUT):
    """Build the Bass program. T seq len, CIN input channels, COUT out channels."""
    assert T % 512 == 0 and CIN % 128 == 0 and COUT % 512 == 0
    KC = CIN // 128        # c_in chunks
    NTB = T // 512         # projection t-blocks == tq blocks
    NQ = T // 512
    NKC = T // 128         # tk chunks
    NCO = COUT // 512      # out-proj column halves

    nc = bacc.Bacc()

    xT = nc.dram_tensor("xT", [CIN, T], F32, kind="ExternalInput")
    Wq = nc.dram_tensor("Wq", [CIN, CH], F32, kind="ExternalInput")
    Wk = nc.dram_tensor("Wk", [CIN, CH], F32, kind="ExternalInput")
    Wv = nc.dram_tensor("Wv", [CIN, CH], F32, kind="ExternalInput")
    Wo = nc.dram_tensor("Wo", [CH, COUT], F32, kind="ExternalInput")
    COS = nc.dram_tensor("COS", [128, T], F32, kind="ExternalInput")
    SIN = nc.dram_tensor("SIN", [128, T], F32, kind="ExternalInput")
    MASK = nc.dram_tensor("MASK", [128, 128], F32, kind="ExternalInput")
    SELA = nc.dram_tensor("SELA", [128, 8], F32, kind="ExternalInput")
    SELB = nc.dram_tensor("SELB", [128, 8], F32, kind="ExternalInput")
    SELTA = nc.dram_tensor("SELTA", [8, 128], F32, kind="ExternalInput")
    SELTB = nc.dram_tensor("SELTB", [8, 128], F32, kind="ExternalInput")
    ONESF = nc.dram_tensor("ONESF", [128, 128], F32, kind="ExternalInput")
    OUT = nc.dram_tensor("OUT", [T, COUT], F32, kind="ExternalOutput")

    xT3 = xT.ap().rearrange("(ko ki) t -> ki ko t", ki=128)      # [128, KC, T]
    Wq3 = Wq.ap().rearrange("(ko ki) m -> ki ko m", ki=128)      # [128, KC, 512]
    Wk3 = Wk.ap().rearrange("(ko ki) m -> ki ko m", ki=128)
    Wv3 = Wv.ap().rearrange("(ko ki) m -> ki ko m", ki=128)
    Wo3 = Wo.ap().rearrange("(mo mi) n -> mi mo n", mi=128)      # [128, 4, COUT]

    with tile.TileContext(nc) as tc:
        with (
            tc.tile_pool(name="consts", bufs=1) as cpool,
            tc.tile_pool(name="big", bufs=1) as big,
            tc.tile_pool(name="w", bufs=1) as wpool,
            tc.tile_pool(name="xtb", bufs=2) as xpool,
            tc.tile_pool(name="work", bufs=2) as work,
            tc.tile_pool(name="tmp", bufs=2) as tmp,
            tc.tile_pool(name="ps4", bufs=1, space="PSUM") as ps4,
            tc.tile_pool(name="psy", bufs=4, space="PSUM") as psy,
        ):
            # ---- constants ----
            mask_sb = cpool.tile([128, 128], F32, tag="mask")
            nc.sync.dma_start(out=mask_sb, in_=MASK[:, :])
            sela_sb = cpool.tile([128, 8], F32R, tag="sela")
            nc.sync.dma_start(out=sela_sb, in_=SELA[:, :].bitcast(F32R))
            selb_sb = cpool.tile([128, 8], F32R, tag="selb")
            nc.sync.dma_start(out=selb_sb, in_=SELB[:, :].bitcast(F32R))
            selta_sb = cpool.tile([8, 128], F32R, tag="selta")
            nc.sync.dma_start(out=selta_sb, in_=SELTA[:, :].bitcast(F32R))
            seltb_sb = cpool.tile([8, 128], F32R, tag="seltb")
            nc.sync.dma_start(out=seltb_sb, in_=SELTB[:, :].bitcast(F32R))
            ones_sb = cpool.tile([128, 64], F32R, tag="ones")
            nc.sync.dma_start(out=ones_sb, in_=ONESF[:, 0:64].bitcast(F32R))
            biasq = cpool.tile([8, 1], F32, tag="biasq")
            nc.vector.memset(biasq, 64.0 * EPS)
            biask = cpool.tile([8, 1], F32, tag="biask")
            nc.vector.memset(biask, EPS)

            khat = big.tile([128, 4, T], F32R, tag="khat")
            vsb = big.tile([128, NKC, 520], F32R, tag="v")
            vsb4 = vsb.rearrange("p n (h e) -> p n h e", e=65)

            def project_qk(w_sb, xtb, dst, ts, is_q, cos_ts=None):
                """Project one 512-t block into dst[:, :, ts] with RoPE+RMS.
                cos_ts: global t slice for the RoPE tables (defaults to ts)."""
                if cos_ts is None:
                    cos_ts = ts
                cos_sb = work.tile([128, 512], F32, tag="cos")
                nc.sync.dma_start(out=cos_sb, in_=COS[:, cos_ts])
                sin_sb = work.tile([128, 512], F32, tag="sin")
                nc.sync.dma_start(out=sin_sb, in_=SIN[:, cos_ts])
                qp = ps4.tile([128, 4, 512], F32, tag="p4", name="qp")
                for cc in range(4):
                    for k in range(KC):
                        nc.tensor.matmul(
                            qp[:, cc, :],
                            w_sb[:, k, 128 * cc:128 * (cc + 1)],
                            xtb[:, k, :],
                            start=(k == 0), stop=(k == KC - 1),
                        )
                # per-head sum of squares (pre-rope == post-rope norms)
                qsq = tmp.tile([128, 4, 512], F32R, tag="qsq", bufs=1)
                nc.scalar.activation(qsq, qp, ACTF.Square)
                ssq = psy.tile([8, 512], F32, tag="y", name="ssq")
                for cc in range(4):
                    nc.tensor.matmul(
                        ssq,
                        sela_sb if cc < 2 else selb_sb,
                        qsq[:, cc, :],
                        start=(cc == 0), stop=(cc == 3),
                    )
                # rms factor rows [8, 512] via exp(-0.5*ln(.)) — keeps ACT on
                # the natural_log_exp table set (same set as attention's Exp)
                sq = tmp.tile([8, 512], F32, tag="sq")
                if is_q:  # 1/sqrt(ssq + 64 eps): folds the 1/sqrt(D) scale
                    nc.scalar.activation(sq, ssq, ACTF.Ln, bias=biasq, scale=1.0)
                else:     # 1/sqrt(ssq/64 + eps)
                    nc.scalar.activation(sq, ssq, ACTF.Ln, bias=biask,
                                         scale=1.0 / 64.0)
                rr = tmp.tile([8, 512], F32R, tag="rr")
                nc.scalar.activation(rr, sq, ACTF.Exp, scale=-0.5)
                # rope + rms scale, per chunk pair (same heads in both chunks)
                for pr in range(2):
                    bq = psy.tile([128, 512], F32, tag="y", name=f"bq{pr}")
                    nc.tensor.matmul(
                        bq, selta_sb if pr == 0 else seltb_sb, rr,
                        start=True, stop=True,
                    )
                    cb = tmp.tile([128, 512], F32, tag="r512", bufs=4)
                    sb_ = tmp.tile([128, 512], F32, tag="r512", bufs=4)
                    nc.vector.tensor_mul(cb, cos_sb, bq)
                    nc.vector.tensor_mul(sb_, sin_sb, bq)
                    u1 = qp[:, 2 * pr, :]
                    u2 = qp[:, 2 * pr + 1, :]
                    e1 = tmp.tile([128, 512], F32, tag="r512", bufs=4)
                    e2 = tmp.tile([128, 512], F32, tag="r512", bufs=4)
                    nc.vector.tensor_mul(e1, u1, cb)
                    nc.vector.tensor_mul(e2, u2, sb_)
                    nc.vector.tensor_add(dst[:, 2 * pr, ts], e1, e2)
                    e3 = tmp.tile([128, 512], F32, tag="r512", bufs=4)
                    e4 = tmp.tile([128, 512], F32, tag="r512", bufs=4)
                    nc.vector.tensor_mul(e3, u2, cb)
                    nc.vector.tensor_mul(e4, u1, sb_)
                    nc.vector.tensor_sub(dst[:, 2 * pr + 1, ts], e3, e4)

            # ============ Phase A: K-hat + V (per t-block) ============
            nc.sync.dma_start(
                out=vsb4[:, :, :, 64],
                in_=ONESF.ap()[:, 0:8 * NKC].rearrange(
                    "p (n h) -> p n h", h=8).bitcast(F32R))
            wk_sb = wpool.tile([128, KC, 512], F32R, tag="wa")
            nc.sync.dma_start(out=wk_sb, in_=Wk3[:, :, :].bitcast(F32R))
            wv_sb = wpool.tile([128, KC, 512], F32R, tag="wb")
            nc.sync.dma_start(out=wv_sb, in_=Wv3[:, :, :].bitcast(F32R))
            for tb in range(NTB):
                ts = slice(512 * tb, 512 * (tb + 1))
                xtb = xpool.tile([128, KC, 512], F32R, tag="xtb")
                nc.sync.dma_start(out=xtb, in_=xT3[:, :, ts].bitcast(F32R))
                project_qk(wk_sb, xtb, khat, ts, is_q=False)
                for j in range(4):
                    vp = psy.tile([128, 512], F32, tag="y", name=f"vp{tb}_{j}")
                    for k in range(KC):
                        nc.tensor.matmul(
                            vp,
                            xtb[:, k, 128 * j:128 * (j + 1)],
                            wv_sb[:, k, :],
                            start=(k == 0), stop=(k == KC - 1),
                        )
                    nc.vector.tensor_copy(
                        out=vsb4[:, 4 * tb + j, :, 0:64],
                        in_=vp.rearrange("p (h d) -> p h d", d=64))

            # ============ Phase B: per tq block: Q-hat, attention, out ====
            wq_sb = wpool.tile([128, KC, 512], F32R, tag="wa")
            nc.sync.dma_start(out=wq_sb, in_=Wq3[:, :, :].bitcast(F32R))
            wo_sb = wpool.tile([128, 4, COUT], F32R, tag="wb")
            nc.sync.dma_start(out=wo_sb, in_=Wo3[:, :, :].bitcast(F32R))

            for qb in range(NQ):
                tqs = slice(512 * qb, 512 * (qb + 1))
                qtb = work.tile([128, 4, 512], F32R, tag="qtb")
                xtb = xpool.tile([128, KC, 512], F32R, tag="xtb", name="xtbq")
                nc.sync.dma_start(out=xtb, in_=xT3[:, :, tqs].bitcast(F32R))
                project_qk(wq_sb, xtb, qtb, slice(0, 512), is_q=True,
                           cos_ts=tqs)

                yhat = work.tile([128, 4, 512], F32R, tag="yhat", bufs=1)
                for g in range(2):
                    ybank = [psy.tile([65, 512], F32, tag="y",
                                      name=f"y{qb}_{g}_{j_}") for j_ in range(4)]
                    nkc = 4 * (qb + 1)
                    for c in range(nkc):
                        sc = ps4.tile([128, 4, 512], F32, tag="p4", name="sc")
                        for j in range(4):
                            for half in range(2):
                                cc = 2 * g + half
                                nc.tensor.matmul(
                                    sc[:, j, :],
                                    khat[32 * j:32 * (j + 1), cc,
                                         128 * c:128 * (c + 1)],
                                    qtb[32 * j:32 * (j + 1), cc, :],
                                    start=(half == 0), stop=(half == 1),
                                    tile_position=(32 * j, 0),
                                )
                        ph = work.tile([128, 4, 512], F32R, tag="phat")
                        nc.scalar.activation(ph, sc, ACTF.Exp)
                        kd = c - 4 * qb
                        if kd >= 0:  # diagonal chunk: causal mask
                            if kd > 0:  # cols [0, 128*kd) fully masked
                                nc.vector.tensor_scalar_mul(
                                    ph[:, :, 0:128 * kd],
                                    ph[:, :, 0:128 * kd], 0.0)
                            nc.vector.tensor_mul(
                                ph[:, :, 128 * kd:128 * (kd + 1)],
                                ph[:, :, 128 * kd:128 * (kd + 1)],
                                mask_sb[:, None, :].to_broadcast([128, 4, 128]),
                            )
                        first, last = (c == 0), (c == nkc - 1)
                        for j in range(4):
                            hloc = 4 * g + j
                            nc.tensor.matmul(
                                ybank[j],
                                vsb[:, c, 65 * hloc:65 * hloc + 65],
                                ph[:, j, :],
                                start=first, stop=last,
                                skip_group_check=True,
                            )
                    # normalize: yhat rows = y / denom
                    for j in range(4):
                        hloc = 4 * g + j
                        rcp = tmp.tile([128, 512], F32R, tag="s512")
                        with nc.allow_low_precision(reason="f32r for PE bcast"):
                            nc.vector.reciprocal(rcp[64:65, :],
                                                 ybank[j][64:65, :])
                        rb = ps4.tile([128, 512], F32, tag="p4",
                                      name=f"rb{qb}_{g}_{j}")
                        nc.tensor.matmul(
                            rb[0:64, :],
                            ones_sb[64:65, :],
                            rcp[64:65, :],
                            start=True, stop=True,
                            tile_position=(64, 0),
                            skip_group_check=True,
                        )
                        rbs = tmp.tile([128, 512], F32, tag="s512")
                        nc.vector.tensor_copy(out=rbs[0:64, :], in_=rb[0:64, :])
                        nc.vector.tensor_mul(
                            yhat[64 * (hloc % 2):64 * (hloc % 2 + 1),
                                 hloc // 2, :],
                            ybank[j][0:64, :],
                            rbs[0:64, :],
                        )
                # out projection for this tq block
                for n in range(NCO):
                    for jt in range(4):
                        op = psy.tile([128, 512], F32, tag="y",
                                      name=f"op{qb}_{n}_{jt}")
                        for m in range(4):
                            nc.tensor.matmul(
                                op,
                                yhat[:, m, 128 * jt:128 * (jt + 1)],
                                wo_sb[:, m, 512 * n:512 * (n + 1)],
                                start=(m == 0), stop=(m == 3),
                            )
                        osb = tmp.tile([128, 512], F32, tag="s512")
                        nc.vector.tensor_copy(out=osb, in_=op)
                        nc.sync.dma_start(
                            out=OUT[512 * qb + 128 * jt:512 * qb + 128 * (jt + 1),
                                    512 * n:512 * (n + 1)],
                            in_=osb)

    nc.finalize()
    return nc


# ======================================================================
# Full-problem harness: 8 cores = 4 batch x 2 head-groups
# ======================================================================
B_FULL, T_FULL, C_FULL, H_FULL = 4, 2048, 1024, 16

_NC_CACHE = {}


def _get_nc():
    if "nc" not in _NC_CACHE:
        _NC_CACHE["nc"] = build_nc(T_FULL, C_FULL, C_FULL)
    return _NC_CACHE["nc"]


def _consts_from_tables(cos, sin):
    """Like make_consts but using the provided RoPE tables.
    cos/sin: [1, 1, T, 32] float32."""
    c = make_consts(T_FULL)
    c["COS"] = np.ascontiguousarray(np.tile(np.asarray(cos)[0, 0].T, (4, 1)))
    c["SIN"] = np.ascontiguousarray(np.tile(np.asarray(sin)[0, 0].T, (4, 1)))
    return c


def make_in_maps(x, cos, sin, Wq, Wk, Wv, Wo):
    x, Wq, Wk, Wv, Wo = (np.asarray(a, dtype=np.float32)
                         for a in (x, Wq, Wk, Wv, Wo))
    consts = _consts_from_tables(cos, sin)
    in_maps = []
    for core in range(8):
        b, hg = core // 2, core % 2
        cols = slice(512 * hg, 512 * (hg + 1))
        in_maps.append(make_core_inputs(
            x[b], Wq[:, cols], Wk[:, cols], Wv[:, cols], Wo[cols, :], consts))
    return in_maps


def gather_out(results):
    out = np.empty((B_FULL, T_FULL, C_FULL), dtype=np.float32)
    for b in range(B_FULL):
        out[b] = results[2 * b]["OUT"] + results[2 * b + 1]["OUT"]
    return out


def kernel(x, cos, sin, Wq, Wk, Wv, Wo):
    from concourse.bass_utils import run_bass_kernel_spmd
    nc = _get_nc()
    in_maps = make_in_maps(x, cos, sin, Wq, Wk, Wv, Wo)
    res = run_bass_kernel_spmd(nc, in_maps, core_ids=list(range(8)))
    return gather_out(res.results)
